# revision 41
# baseline (speedup 1.0000x reference)
"""ComplexLayerNorm Trainium2 kernel (8 NeuronCores, SPMD, F-sharded).

Math (see reference): per-feature 2x2 covariance whitening of (re, im) over
all B*C samples (centered with the batch-only mean mu_b), after subtracting
the complex mean over F, plus complex affine.

v4 design:
  * F-sharding: each core owns 256 features (2 chunks of 128 on partitions)
    and ALL B*C = 8192 samples; the covariance path is fully core-local.
  * Only the per-sample complex mean over F crosses cores: partials are
    scaled x64, quantized to fp8e4 (plenty for a mean-subtraction term) and
    ride ONE AllGather that overlaps the apply phase.  The 8-way shard sum
    AND the beta add are folded into a K=17 correction matmul (16 gathered
    mean rows + a ones row), whose rhs carries -A/64 coefficients.
  * Engine assignment respects the cost model: ACT does Square+accum and
    the PSUM mean-staging copies; Pool does the cross products and the
    first T-tree fold (TensorTensor); DVE does the x_i^2 products (2x
    fp16 TT), all second-moment accumulations (4x fp16 TensorScalarPtr
    accum), and the deep T-tree folds.
  * Two-pass apply: pass 1 (collective-independent) does the diagonal-W
    x-matmuls into PSUM and stages uncorrected A@x; pass 2 adds the K=17
    correction (DVE stt from PSUM on even groups, ACT-copy + Pool add on
    odd groups) and stores.  Emission interleaves the passes so the PE
    never idles waiting for the collective.
"""

import numpy as np
import ml_dtypes

import bass_rust
import concourse.bass as bass
import concourse.mybir as mybir
from concourse import tile
from concourse.bass_utils import run_bass_kernel_spmd


def split_multi_waits(nc):
    """The walrus build in this container allows only ONE sync-wait command
    per instruction; Tile emits several.  Split extras into preceding
    single-wait NoOps on the same engine (sequential waits == AND)."""
    cnt = 0
    for bb in nc.main_func.blocks:
        il = bb.instructions
        newlist = []
        changed = False
        for inst in list(il):
            si = inst.sync_info
            waits = list(si.on_wait) if si else []
            if len(waits) > 1:
                changed = True
                for w in waits[:-1]:
                    cnt += 1
                    nop = bass_rust.InstNoOp(name=f"I-wsplit-{cnt}")
                    nop.engine = inst.engine
                    nop.sync_info = mybir.SyncInfo(on_wait=[w], on_update=[])
                    newlist.append(nop)
                inst.sync_info = mybir.SyncInfo(
                    on_wait=[waits[-1]], on_update=list(si.on_update))
            newlist.append(inst)
        if changed:
            il[:] = newlist
    return cnt

FP = mybir.dt.float32
FR = mybir.dt.float32r
F16 = mybir.dt.float16
F8 = mybir.dt.float8e4
AF = mybir.ActivationFunctionType
OP = mybir.AluOpType
AX = mybir.AxisListType

B, C, F = 64, 128, 2048
NCORES = 8
FSH = F // NCORES           # 256 features per core
NCH = FSH // 128            # 2 f-chunks of 128 (on partitions)
BC = B * C                  # 8192 samples per core (full batch)
NBB = 4                     # bigblocks of 2048 samples for DMA streaming
BBS = BC // NBB             # 2048
NG = BC // 128              # 64 apply groups (128 samples each)
EPS = 1e-4
NM1 = float(B * C - 1)      # 8191
MSCALE = 64.0               # fp8 mean-partial scaling (folded into consts)


def build_bass():
    nc = bass.Bass()

    # x, fp16, f-on-partitions: xt[p, 16384*comp + 8192*cc + j]
    #   = x_comp[sample j, f_local = 128*cc + p]
    xt = nc.dram_tensor("xt", [128, 2 * NCH * BC], F16, kind="ExternalInput")
    # merged constants: cf32 = [ident | g_r | g_i], cf16 = [onesF*MSCALE/F |
    # onesEv | onesOd]
    cf32 = nc.dram_tensor("cf32", [128, 128 + 2 * NCH], FP,
                          kind="ExternalInput")
    cf16 = nc.dram_tensor("cf16", [128, 40], F16, kind="ExternalInput")
    # beta for this shard (fp8), apply-column order: [0, 256*cc + 2*g + c]
    beta_row = nc.dram_tensor("beta_row", [1, 2 * FSH], F8,
                              kind="ExternalInput")
    onesrow = nc.dram_tensor("onesrow", [1, BC], F8, kind="ExternalInput")

    out = nc.dram_tensor("out", [BC, 2 * FSH], F16, kind="ExternalOutput")

    with tile.TileContext(nc) as tc:
        with (
            tc.tile_pool(name="big", bufs=1) as big,
            tc.tile_pool(name="small", bufs=1) as small,
            tc.tile_pool(name="stg", bufs=1) as stgp,
            tc.tile_pool(name="dram", bufs=1, space="DRAM") as dram,
        ):
            # ---- constants (2 merged DMAs, issued after the first x
            # block on the scalar queue)
            cf32_t = small.tile([128, 128 + 2 * NCH], FP, tag="cf32")
            cf16_t = small.tile([128, 40], F16, tag="cf16")
            ident_t = cf32_t[:, 0:128]
            g_r_t = cf32_t[:, 128:128 + NCH]
            g_i_t = cf32_t[:, 128 + NCH:128 + 2 * NCH]
            onesF_t = cf16_t[:, 0:1]
            onesEv_t = cf16_t[:, 8:24]
            onesOd_t = cf16_t[:, 24:40]

            def emit_const_dmas():
                nc.scalar.dma_start(cf16_t[:], cf16[:])
                nc.scalar.dma_start(cf32_t[:], cf32[:])

            # mg: 16 gathered fp8 mean-partial rows + ones row (for beta)
            mg = small.tile([17, BC], F8, tag="mg")
            # Arow17: correction-matmul rhs (fp8); row 16 = beta
            Arow = small.tile([17, 2 * FSH], F8, tag="Arow")

            # ---- persistent x (fp16, f-on-partitions)
            xT = big.tile([128, 2 * NCH * BC], F16, tag="xT")

            # batch-sums over b: T_sb[p, 128*(2*cc+comp) + c]
            T_sb = small.tile([128, 2 * NCH * C], F16, tag="T_sb")
            # second-moment accumulator columns: 8 cols (cc,b) per
            # moment m (0=rr, 1=ii, 2=ri)
            S_acc = small.tile([128, 24], FP, tag="S_acc")

            from contextlib import ExitStack
            _stk = ExitStack()
            scr = _stk.enter_context(tc.tile_pool(name="scr", bufs=1))
            sc2 = _stk.enter_context(tc.tile_pool(name="sc2", bufs=2))
            trp = _stk.enter_context(tc.tile_pool(name="trp", bufs=2))
            msp = _stk.enter_context(tc.tile_pool(name="msp", bufs=2))
            ps_mean = _stk.enter_context(
                tc.tile_pool(name="ps_mean", bufs=2, space="PSUM"))
            ar_in = dram.tile([2, BC], F8, tag="ar_in")
            ar_out = dram.tile([16, BC], F8, tag="ar_out")

            def xsl(comp, cc, lo, n):
                return xT[:, 16384 * comp + BC * cc + lo:
                          16384 * comp + BC * cc + lo + n]

            def xdr(comp, cc, lo, n):
                return xt[:, 16384 * comp + BC * cc + lo:
                          16384 * comp + BC * cc + lo + n]

            # ---- Phase A: stream bigblocks; stats overlap the DMA.
            # The last bigblock is special-cased so the mean-partial chain
            # (matmuls -> ACT copies -> drains -> AllGather) issues before
            # its trailing stats work.
            prod_i = {}

            def emit_dmas(b, cc):
                lo = BBS * b
                for comp in range(2):
                    nc.sync.dma_start(xsl(comp, cc, lo, BBS),
                                      xdr(comp, cc, lo, BBS))

            def emit_sq_acc(b, cc, act_only):
                lo = BBS * b
                xr = xsl(0, cc, lo, BBS)
                xi = xsl(1, cc, lo, BBS)
                sa = scr.tile([128, BBS], F16, tag="sq_act")
                nc.scalar.activation(
                    sa[:], xr, AF.Square,
                    accum_out=S_acc[:, cc * 4 + b:cc * 4 + b + 1])
                if act_only:
                    sb_ = scr.tile([128, BBS], F16, tag="sq_act",
                                   name=f"sq_i_{cc}_{b}")
                    nc.scalar.activation(
                        sb_[:], xi, AF.Square,
                        accum_out=S_acc[:, 8 + cc * 4 + b:
                                        8 + cc * 4 + b + 1])
                else:
                    pi = scr.tile([128, BBS], F16, tag="prod_i",
                                  name=f"prod_i_{cc}_{b}")
                    nc.vector.tensor_tensor(out=pi[:], in0=xi, in1=xi,
                                            op=OP.mult)
                    nc.vector.tensor_scalar(
                        out=pi[:], in0=pi[:], scalar1=1.0, scalar2=0.0,
                        op0=OP.mult, op1=OP.add,
                        accum_out=S_acc[:, 8 + cc * 4 + b:
                                        8 + cc * 4 + b + 1])
                px = sc2.tile([128, BBS], F16, tag="prod_x")
                nc.gpsimd.tensor_tensor(out=px[:], in0=xr, in1=xi,
                                        op=OP.mult)
                nc.vector.tensor_scalar(
                    out=px[:], in0=px[:], scalar1=1.0, scalar2=0.0,
                    op0=OP.mult, op1=OP.add,
                    accum_out=S_acc[:, 16 + cc * 4 + b:
                                    16 + cc * 4 + b + 1])

            def emit_means(b, cc):
                lo = BBS * b
                for comp in range(2):
                    if cc == 0 and comp == 0:
                        prod_i[("pm", b)] = [
                            ps_mean.tile([128, 1024], FP, tag=f"psm{c2}",
                                         name=f"psm{c2}_{b}")
                            for c2 in range(2)]
                    pm = prod_i[("pm", b)]
                    for m in range(4):
                        nc.tensor.matmul(
                            pm[comp][64 * (m % 2):64 * (m % 2) + 1,
                                     512 * (m // 2):512 * (m // 2) + 512],
                            onesF_t,
                            xsl(comp, cc, lo + 512 * m, 512),
                            start=(cc == 0), stop=(cc == NCH - 1))

            def emit_tree(b, cc, comp, deep_pool):
                lo = BBS * b
                t1 = trp.tile([128, BBS // 2], F16, tag="t1")
                nc.gpsimd.tensor_tensor(
                    out=t1[:], in0=xsl(comp, cc, lo, BBS // 2),
                    in1=xsl(comp, cc, lo + BBS // 2, BBS // 2), op=OP.add)
                eng = nc.gpsimd if deep_pool else nc.vector
                t2 = trp.tile([128, BBS // 4], F16, tag="t2")
                eng.tensor_tensor(out=t2[:], in0=t1[:, 0:BBS // 4],
                                  in1=t1[:, BBS // 4:BBS // 2], op=OP.add)
                t3 = trp.tile([128, BBS // 8], F16, tag="t3")
                eng.tensor_tensor(out=t3[:], in0=t2[:, 0:BBS // 8],
                                  in1=t2[:, BBS // 8:BBS // 4], op=OP.add)
                td = T_sb[:, C * (2 * cc + comp):C * (2 * cc + comp) + C]
                if b == 0:
                    eng.tensor_tensor(out=td, in0=t3[:, 0:C],
                                      in1=t3[:, C:2 * C], op=OP.add)
                else:
                    t4 = trp.tile([128, C], F16, tag="t4")
                    eng.tensor_tensor(out=t4[:], in0=t3[:, 0:C],
                                      in1=t3[:, C:2 * C], op=OP.add)
                    eng.tensor_tensor(out=td, in0=td, in1=t4[:], op=OP.add)

            def emit_stage_drain(b):
                pm = prod_i[("pm", b)]
                mt = msp.tile([128, 2048], F8, tag="ms", name=f"ms_{b}")
                nc.scalar.copy(mt[:, 0:1024], pm[0][:])
                nc.scalar.copy(mt[:, 1024:2048], pm[1][:])
                for comp in range(2):
                    srcd = mt[:, 1024 * comp:1024 * (comp + 1)].rearrange(
                        "(m o) (h j) -> m o h j", o=64, h=2)[:, 0:1]
                    dstd = ar_in[comp:comp + 1,
                                 BBS * b:BBS * (b + 1)].rearrange(
                        "o (h m j) -> m o h j", h=2, m=2)
                    nc.sync.dma_start(dstd, srcd)

            def emit_collective():
                nc.gpsimd.collective_compute(
                    "AllGather", OP.bypass,
                    replica_groups=[list(range(NCORES))],
                    ins=[ar_in.opt()],
                    outs=[ar_out.opt()],
                )
                # pass-2 constants ride the collective window
                nc.scalar.dma_start(mg[16:17, :], onesrow[:])
                nc.scalar.dma_start(Arow[16:17, :], beta_row[:])
                for hf in range(2):
                    nc.sync.dma_start(
                        mg[0:16, 4096 * hf:4096 * (hf + 1)],
                        ar_out[:, 4096 * hf:4096 * (hf + 1)])

            for b in range(NBB):
                last = b == NBB - 1
                for cc in range(NCH):
                    emit_dmas(b, cc)
                    if b == 0 and cc == 0:
                        emit_const_dmas()
                    emit_means(b, cc)
                    if not (last and cc == 1):
                        emit_sq_acc(b, cc, act_only=False)
                emit_stage_drain(b)
                if last:
                    emit_collective()
                    emit_sq_acc(b, 1, act_only=True)
                for cc in range(NCH):
                    for comp in range(2):
                        emit_tree(b, cc, comp, deep_pool=(last and comp == 1))

            # ---- stats finalize: S6 (cols m*2+cc), corr6, cov6
            S6 = small.tile([128, 6], FP, tag="S6")
            nc.vector.tensor_reduce(
                S6[:, 0:2], S_acc[:, 0:8].rearrange("p (g b) -> p g b", b=4),
                AX.X, OP.add)
            nc.vector.tensor_reduce(
                S6[:, 2:6], S_acc[:, 8:24].rearrange("p (g b) -> p g b", b=4),
                AX.X, OP.add)
            corr6 = small.tile([128, 6], FP, tag="corr6")
            for m, (ca, cb) in enumerate(((0, 0), (1, 1), (0, 1))):
                for cc in range(NCH):
                    pr = sc2.tile([128, C], F16, tag="prod_x",
                                  name=f"tt_{m}_{cc}")
                    nc.vector.scalar_tensor_tensor(
                        out=pr[:],
                        in0=T_sb[:, C * (2 * cc + ca):C * (2 * cc + ca) + C],
                        scalar=1.0,
                        in1=T_sb[:, C * (2 * cc + cb):C * (2 * cc + cb) + C],
                        op0=OP.mult, op1=OP.mult,
                        accum_out=corr6[:, m * 2 + cc:m * 2 + cc + 1])
            cov6 = small.tile([128, 6], FP, tag="cov6")
            nc.vector.scalar_tensor_tensor(
                out=cov6[:], in0=corr6[:], scalar=-1.0 / B, in1=S6[:],
                op0=OP.mult, op1=OP.add)
            nc.vector.tensor_scalar(
                out=cov6[:], in0=cov6[:], scalar1=1.0 / NM1, scalar2=None,
                op0=OP.mult)
            _stk.close()  # release scratch + mean PSUM

            # ---- Phase C: closed-form 2x2 inverse sqrt, fold gamma
            def stile(tag):
                return small.tile([128, NCH], FP, tag=tag, name=tag)

            arr, cii = stile("arr"), stile("cii")
            bri = cov6[:, 4:6]
            nc.vector.tensor_scalar(out=arr[:], in0=cov6[:, 0:2],
                                    scalar1=EPS, scalar2=None, op0=OP.add)
            nc.vector.tensor_scalar(out=cii[:], in0=cov6[:, 2:4],
                                    scalar1=EPS, scalar2=None, op0=OP.add)

            det, tmp = stile("det"), stile("tmp")
            tsum0 = stile("tsum0")
            nc.vector.tensor_tensor(out=tmp[:], in0=bri, in1=bri,
                                    op=OP.mult)
            nc.vector.tensor_tensor(out=det[:], in0=arr[:], in1=cii[:],
                                    op=OP.mult)
            nc.vector.tensor_tensor(out=tsum0[:], in0=arr[:], in1=cii[:],
                                    op=OP.add)
            nc.vector.tensor_tensor(out=det[:], in0=det[:], in1=tmp[:],
                                    op=OP.subtract)
            s_t = stile("s_t")
            nc.scalar.activation(s_t[:], det[:], AF.Sqrt)
            tsum = stile("tsum")
            nc.vector.scalar_tensor_tensor(out=tsum[:], in0=s_t[:], scalar=2.0,
                                           in1=tsum0[:], op0=OP.mult,
                                           op1=OP.add)
            tval = stile("tval")
            nc.scalar.activation(tval[:], tsum[:], AF.Sqrt)
            den, rden = stile("den"), stile("rden")
            nc.vector.tensor_tensor(out=den[:], in0=s_t[:], in1=tval[:],
                                    op=OP.mult)
            nc.vector.reciprocal(rden[:], den[:])

            w_rr, w_ii, wri = stile("w_rr"), stile("w_ii"), stile("wri")
            nc.vector.tensor_tensor(out=w_rr[:], in0=cii[:], in1=s_t[:],
                                    op=OP.add)  # runs parallel to tval chain
            nc.vector.tensor_tensor(out=w_rr[:], in0=w_rr[:], in1=rden[:],
                                    op=OP.mult)
            nc.vector.tensor_tensor(out=w_ii[:], in0=arr[:], in1=s_t[:],
                                    op=OP.add)
            nc.vector.tensor_tensor(out=w_ii[:], in0=w_ii[:], in1=rden[:],
                                    op=OP.mult)
            nc.vector.scalar_tensor_tensor(out=wri[:], in0=bri,
                                           scalar=-1.0, in1=rden[:],
                                           op0=OP.mult, op1=OP.mult)

            # A = G @ W
            a_rr, a_ri = stile("a_rr"), stile("a_ri")
            a_ir, a_ii = stile("a_ir"), stile("a_ii")
            u, v = stile("u"), stile("v")
            for dst, (wa, wb) in ((a_rr, (w_rr, wri)), (a_ri, (wri, w_ii))):
                nc.vector.tensor_tensor(out=u[:], in0=g_r_t, in1=wa[:],
                                        op=OP.mult)
                nc.vector.tensor_tensor(out=v[:], in0=g_i_t, in1=wb[:],
                                        op=OP.mult)
                nc.vector.tensor_tensor(out=dst[:], in0=u[:], in1=v[:],
                                        op=OP.subtract)
            for dst, (wa, wb) in ((a_ir, (w_rr, wri)), (a_ii, (wri, w_ii))):
                nc.vector.tensor_tensor(out=u[:], in0=g_i_t, in1=wa[:],
                                        op=OP.mult)
                nc.vector.tensor_tensor(out=v[:], in0=g_r_t, in1=wb[:],
                                        op=OP.mult)
                nc.vector.tensor_tensor(out=dst[:], in0=u[:], in1=v[:],
                                        op=OP.add)

            # ---- W tiles (fp16), diagonal per chunk: W[p, 2g+c]
            Ws = []
            for cc in range(NCH):
                W_r = small.tile([128, 256], F16, tag=f"W_r{cc}",
                                 name=f"W_r{cc}")
                W_i = small.tile([128, 256], F16, tag=f"W_i{cc}",
                                 name=f"W_i{cc}")
                for W, (ev, od) in ((W_r, (a_rr, a_ir)), (W_i, (a_ri, a_ii))):
                    Wv = W[:].rearrange("p (g c) -> p g c", c=2)
                    nc.vector.tensor_scalar(
                        out=Wv[:, :, 0], in0=ident_t,
                        scalar1=ev[:, cc:cc + 1], scalar2=None, op0=OP.mult)
                    nc.vector.tensor_scalar(
                        out=Wv[:, :, 1], in0=ident_t,
                        scalar1=od[:, cc:cc + 1], scalar2=None, op0=OP.mult)
                Ws.append((W_r, W_i))

            # ---- Arow rows 0..15: -A/MSCALE coefs in apply-column order,
            # via matmuls against the W tiles with even/odd selectors.
            from contextlib import ExitStack as _ES2
            _stk2 = _ES2()
            ps_t = _stk2.enter_context(
                tc.tile_pool(name="ps_t", bufs=1, space="PSUM"))
            psA = ps_t.tile([16, 2 * FSH], FP, tag="psA")
            for cc in range(NCH):
                W_r, W_i = Ws[cc]
                nc.tensor.matmul(psA[:, 256 * cc:256 * (cc + 1)],
                                 onesEv_t, W_r[:], start=True, stop=False)
                nc.tensor.matmul(psA[:, 256 * cc:256 * (cc + 1)],
                                 onesOd_t, W_i[:], start=False, stop=True)
            nc.vector.tensor_scalar(out=Arow[0:16, :], in0=psA[:],
                                    scalar1=-1.0 / MSCALE, scalar2=None,
                                    op0=OP.mult)
            _stk2.close()

            # ---- Phase D: two-pass apply with interleaved emission
            stg = stgp.tile([128, NG * 512], F16, tag="stg")
            _stk3 = _ES2()
            ps_o = _stk3.enter_context(
                tc.tile_pool(name="ps_o", bufs=7, space="PSUM"))
            corrp = _stk3.enter_context(tc.tile_pool(name="corrp", bufs=2))

            def pass1(g):
                po = ps_o.tile([128, 512], FP, tag="po", name=f"po1_{g}")
                for cc in range(NCH):
                    W_r, W_i = Ws[cc]
                    nc.tensor.matmul(
                        po[:, 256 * cc:256 * (cc + 1)],
                        xsl(0, cc, 128 * g, 128), W_r[:],
                        start=True, stop=False)
                    nc.tensor.matmul(
                        po[:, 256 * cc:256 * (cc + 1)],
                        xsl(1, cc, 128 * g, 128), W_i[:],
                        start=False, stop=True)
                dst = stg[:, 512 * g:512 * (g + 1)]
                if g % 2 == 0:
                    nc.vector.tensor_copy(dst, po[:])
                else:
                    nc.scalar.copy(dst, po[:])

            def store4(g):
                g0 = g - 3
                dstd = out.rearrange("(a p) f -> p a f", p=128)[
                    :, g0:g0 + 4, :]
                srcd = stg[:, 512 * g0:512 * (g + 1)].rearrange(
                    "p (a q) -> p a q", q=512)
                if (g // 4) % 2 == 0:
                    nc.sync.dma_start(dstd, srcd)
                else:
                    nc.scalar.dma_start(dstd, srcd)

            def single(g):
                # one-pass group: K17 correction first (full region, start),
                # then the x-matmul sub-region accumulations (baseline's
                # beta-first PSUM pattern).
                po = ps_o.tile([128, 512], FP, tag="po", name=f"po1_{g}")
                nc.tensor.matmul(
                    po[:], mg[:, 128 * g:128 * (g + 1)], Arow[:],
                    start=True, stop=False)
                for cc in range(NCH):
                    W_r, W_i = Ws[cc]
                    nc.tensor.matmul(
                        po[:, 256 * cc:256 * (cc + 1)],
                        xsl(0, cc, 128 * g, 128), W_r[:],
                        start=False, stop=False)
                    nc.tensor.matmul(
                        po[:, 256 * cc:256 * (cc + 1)],
                        xsl(1, cc, 128 * g, 128), W_i[:],
                        start=False, stop=(cc == NCH - 1))
                dst = stg[:, 512 * g:512 * (g + 1)]
                if g % 2 == 0:
                    nc.vector.tensor_copy(dst, po[:])
                else:
                    nc.scalar.copy(dst, po[:])
                if g % 4 == 3:
                    store4(g)

            def pass2(g):
                po = ps_o.tile([128, 512], FP, tag="po", name=f"po2_{g}")
                nc.tensor.matmul(
                    po[:], mg[:, 128 * g:128 * (g + 1)], Arow[:],
                    start=True, stop=True)
                dst = stg[:, 512 * g:512 * (g + 1)]
                if g % 2 == 0 or g >= 28:
                    nc.vector.scalar_tensor_tensor(
                        out=dst, in0=dst, scalar=1.0, in1=po[:],
                        op0=OP.mult, op1=OP.add)
                else:
                    ct = corrp.tile([128, 512], F16, tag="ct")
                    nc.scalar.copy(ct[:], po[:])
                    nc.gpsimd.tensor_tensor(
                        out=dst, in0=dst, in1=ct[:], op=OP.add)
                if g % 4 == 3:
                    store4(g)

            SPLIT = 28
            for g in range(SPLIT):
                pass1(g)
            p2 = 0
            for i, g in enumerate(range(SPLIT, NG)):
                single(g)
                want = (i + 1) * SPLIT // (NG - SPLIT)
                while p2 < min(want, SPLIT):
                    pass2(p2)
                    p2 += 1
            while p2 < SPLIT:
                pass2(p2)
                p2 += 1
            _stk3.close()

    split_multi_waits(nc)
    return nc


_CACHE = {}


def _get_nc():
    if "nc" not in _CACHE:
        _CACHE["nc"] = build_bass()
    return _CACHE["nc"]


def _constants():
    if "consts" not in _CACHE:
        cf16 = np.zeros((128, 40), dtype=np.float16)
        cf16[:, 0] = MSCALE / F
        cf16[:, 8:24] = (np.arange(16) % 2 == 0).astype(np.float16)
        cf16[:, 24:40] = (np.arange(16) % 2 == 1).astype(np.float16)
        _CACHE["consts"] = {
            "cf16": cf16,
            "onesrow": np.ones((1, BC), dtype=ml_dtypes.float8_e4m3),
        }
    return _CACHE["consts"]


def _host_xt(xr, xi, fsl):
    """Build xt[p, 16384*comp + 8192*cc + j] = x_comp[j, 128*cc + p]."""
    halves = []
    for x in (xr, xi):
        xs = x[:, fsl].reshape(BC, NCH, 128)        # (j, cc, p)
        halves.append(np.transpose(xs, (2, 1, 0)).reshape(128, NCH * BC))
    return np.ascontiguousarray(
        np.concatenate(halves, axis=1)).astype(np.float16)


def kernel(x_real, x_imag, gamma_r, gamma_i, beta_r, beta_i):
    x_real = np.asarray(x_real, dtype=np.float32).reshape(BC, F)
    x_imag = np.asarray(x_imag, dtype=np.float32).reshape(BC, F)
    gamma_r = np.asarray(gamma_r, dtype=np.float32)
    gamma_i = np.asarray(gamma_i, dtype=np.float32)
    beta_r = np.asarray(beta_r, dtype=np.float32)
    beta_i = np.asarray(beta_i, dtype=np.float32)

    nc = _get_nc()
    consts = _constants()

    in_maps = []
    for k in range(NCORES):
        fsl = slice(FSH * k, FSH * (k + 1))
        cf32 = np.empty((128, 128 + 2 * NCH), dtype=np.float32)
        cf32[:, 0:128] = np.eye(128, dtype=np.float32)
        cf32[:, 128:128 + NCH] = gamma_r[fsl].reshape(NCH, 128).T
        cf32[:, 128 + NCH:] = gamma_i[fsl].reshape(NCH, 128).T
        beta_row = np.ascontiguousarray(
            np.stack([beta_r[fsl], beta_i[fsl]], axis=-1).reshape(1, 2 * FSH)
        ).astype(ml_dtypes.float8_e4m3)
        in_maps.append({
            "xt": _host_xt(x_real, x_imag, fsl),
            "cf32": cf32, "beta_row": beta_row,
            **consts,
        })

    res = run_bass_kernel_spmd(nc, in_maps, list(range(NCORES)))

    full = np.empty((B, C, F, 2), dtype=np.float32)
    for k in range(NCORES):
        full[:, :, FSH * k:FSH * (k + 1)] = (
            np.asarray(res.results[k]["out"]).astype(np.float32)
            .reshape(B, C, FSH, 2)
        )
    return full


# revision 43
# speedup vs baseline: 1.1943x; 1.1943x over previous
"""ComplexLayerNorm Trainium2 kernel (8 NeuronCores, SPMD, F-sharded).

Math (see reference): per-feature 2x2 covariance whitening of (re, im) over
all B*C samples (centered with the batch-only mean mu_b), after subtracting
the complex mean over F, plus complex affine.

v4 design:
  * F-sharding: each core owns 256 features (2 chunks of 128 on partitions)
    and ALL B*C = 8192 samples; the covariance path is fully core-local.
  * Only the per-sample complex mean over F crosses cores: partials are
    scaled x64, quantized to fp8e4 (plenty for a mean-subtraction term) and
    ride ONE AllGather that overlaps the apply phase.  The 8-way shard sum
    AND the beta add are folded into a K=17 correction matmul (16 gathered
    mean rows + a ones row), whose rhs carries -A/64 coefficients.
  * Engine assignment respects the cost model: ACT does Square+accum and
    the PSUM mean-staging copies; Pool does the cross products and the
    first T-tree fold (TensorTensor); DVE does the x_i^2 products (2x
    fp16 TT), all second-moment accumulations (4x fp16 TensorScalarPtr
    accum), and the deep T-tree folds.
  * Two-pass apply: pass 1 (collective-independent) does the diagonal-W
    x-matmuls into PSUM and stages uncorrected A@x; pass 2 adds the K=17
    correction (DVE stt from PSUM on even groups, ACT-copy + Pool add on
    odd groups) and stores.  Emission interleaves the passes so the PE
    never idles waiting for the collective.
"""

import numpy as np
import ml_dtypes

import bass_rust
import concourse.bass as bass
import concourse.mybir as mybir
from concourse import tile
from concourse.bass_utils import run_bass_kernel_spmd


def split_multi_waits(nc):
    """The walrus build in this container allows only ONE sync-wait command
    per instruction; Tile emits several.  Split extras into preceding
    single-wait NoOps on the same engine (sequential waits == AND)."""
    cnt = 0
    for bb in nc.main_func.blocks:
        il = bb.instructions
        newlist = []
        changed = False
        for inst in list(il):
            si = inst.sync_info
            waits = list(si.on_wait) if si else []
            if len(waits) > 1:
                changed = True
                for w in waits[:-1]:
                    cnt += 1
                    nop = bass_rust.InstNoOp(name=f"I-wsplit-{cnt}")
                    nop.engine = inst.engine
                    nop.sync_info = mybir.SyncInfo(on_wait=[w], on_update=[])
                    newlist.append(nop)
                inst.sync_info = mybir.SyncInfo(
                    on_wait=[waits[-1]], on_update=list(si.on_update))
            newlist.append(inst)
        if changed:
            il[:] = newlist
    return cnt

FP = mybir.dt.float32
FR = mybir.dt.float32r
F16 = mybir.dt.float16
F8 = mybir.dt.float8e4
AF = mybir.ActivationFunctionType
OP = mybir.AluOpType
AX = mybir.AxisListType

B, C, F = 64, 128, 2048
NCORES = 8
FSH = F // NCORES           # 256 features per core
NCH = FSH // 128            # 2 f-chunks of 128 (on partitions)
BC = B * C                  # 8192 samples per core (full batch)
NBB = 4                     # bigblocks of 2048 samples for DMA streaming
BBS = BC // NBB             # 2048
NG = BC // 128              # 64 apply groups (128 samples each)
EPS = 1e-4
NM1 = float(B * C - 1)      # 8191
MSCALE = 64.0               # fp8 mean-partial scaling (folded into consts)


def build_bass():
    nc = bass.Bass()

    # x, fp16, f-on-partitions: xt[p, 16384*comp + 8192*cc + j]
    #   = x_comp[sample j, f_local = 128*cc + p]
    xt = nc.dram_tensor("xt", [128, 2 * NCH * BC], F16, kind="ExternalInput")
    # merged constants: cf32 = [ident | g_r | g_i], cf16 = [onesF*MSCALE/F |
    # onesEv | onesOd]
    cf32 = nc.dram_tensor("cf32", [128, 128 + 2 * NCH], FP,
                          kind="ExternalInput")
    cf16 = nc.dram_tensor("cf16", [128, 40], F16, kind="ExternalInput")
    # beta for this shard (fp8), apply-column order: [0, 256*cc + 2*g + c]
    beta_row = nc.dram_tensor("beta_row", [1, 2 * FSH], F8,
                              kind="ExternalInput")
    onesrow = nc.dram_tensor("onesrow", [1, BC], F8, kind="ExternalInput")

    out = nc.dram_tensor("out", [BC, 2 * FSH], F16, kind="ExternalOutput")

    with tile.TileContext(nc) as tc:
        with (
            tc.tile_pool(name="big", bufs=1) as big,
            tc.tile_pool(name="small", bufs=1) as small,
            tc.tile_pool(name="stg", bufs=1) as stgp,
            tc.tile_pool(name="dram", bufs=1, space="DRAM") as dram,
        ):
            # ---- constants (2 merged DMAs, issued after the first x
            # block on the scalar queue)
            cf32_t = small.tile([128, 128 + 2 * NCH], FP, tag="cf32")
            cf16_t = small.tile([128, 40], F16, tag="cf16")
            ident_t = cf32_t[:, 0:128]
            g_r_t = cf32_t[:, 128:128 + NCH]
            g_i_t = cf32_t[:, 128 + NCH:128 + 2 * NCH]
            onesF_t = cf16_t[:, 0:1]
            onesEv_t = cf16_t[:, 8:24]
            onesOd_t = cf16_t[:, 24:40]

            def emit_const_dmas():
                nc.scalar.dma_start(cf16_t[:], cf16[:])
                nc.scalar.dma_start(cf32_t[:], cf32[:])

            # mg: 16 gathered fp8 mean-partial rows + ones row (for beta)
            mg = small.tile([17, BC], F8, tag="mg")
            # Arow17: correction-matmul rhs (fp8); row 16 = beta
            Arow = small.tile([17, 2 * FSH], F8, tag="Arow")

            # ---- persistent x (fp16, f-on-partitions)
            xT = big.tile([128, 2 * NCH * BC], F16, tag="xT")

            # batch-sums over b: T_sb[p, 128*(2*cc+comp) + c]
            T_sb = small.tile([128, 2 * NCH * C], F16, tag="T_sb")
            # second-moment accumulator columns: 8 cols (cc,b) per
            # moment m (0=rr, 1=ii, 2=ri)
            S_acc = small.tile([128, 24], FP, tag="S_acc")

            from contextlib import ExitStack
            _stk = ExitStack()
            scr = _stk.enter_context(tc.tile_pool(name="scr", bufs=1))
            sc2 = _stk.enter_context(tc.tile_pool(name="sc2", bufs=2))
            trp = _stk.enter_context(tc.tile_pool(name="trp", bufs=2))
            msp = _stk.enter_context(tc.tile_pool(name="msp", bufs=2))
            ps_mean = _stk.enter_context(
                tc.tile_pool(name="ps_mean", bufs=2, space="PSUM"))
            ar_in = dram.tile([2, BC], F8, tag="ar_in")
            ar_out = dram.tile([16, BC], F8, tag="ar_out")

            def xsl(comp, cc, lo, n):
                return xT[:, 16384 * comp + BC * cc + lo:
                          16384 * comp + BC * cc + lo + n]

            def xdr(comp, cc, lo, n):
                return xt[:, 16384 * comp + BC * cc + lo:
                          16384 * comp + BC * cc + lo + n]

            # ---- Phase A: stream bigblocks; stats overlap the DMA.
            # The last bigblock is special-cased so the mean-partial chain
            # (matmuls -> ACT copies -> drains -> AllGather) issues before
            # its trailing stats work.
            prod_i = {}

            def emit_dmas(b, cc):
                lo = BBS * b
                for comp in range(2):
                    nc.sync.dma_start(xsl(comp, cc, lo, BBS),
                                      xdr(comp, cc, lo, BBS))

            def emit_sq_acc(b, cc, act_only):
                lo = BBS * b
                xr = xsl(0, cc, lo, BBS)
                xi = xsl(1, cc, lo, BBS)
                sa = scr.tile([128, BBS], F16, tag="sq_act")
                nc.scalar.activation(
                    sa[:], xr, AF.Square,
                    accum_out=S_acc[:, cc * 4 + b:cc * 4 + b + 1])
                if act_only:
                    sb_ = scr.tile([128, BBS], F16, tag="sq_act",
                                   name=f"sq_i_{cc}_{b}")
                    nc.scalar.activation(
                        sb_[:], xi, AF.Square,
                        accum_out=S_acc[:, 8 + cc * 4 + b:
                                        8 + cc * 4 + b + 1])
                else:
                    pi = scr.tile([128, BBS], F16, tag="prod_i",
                                  name=f"prod_i_{cc}_{b}")
                    nc.vector.tensor_tensor(out=pi[:], in0=xi, in1=xi,
                                            op=OP.mult)
                    nc.vector.tensor_scalar(
                        out=pi[:], in0=pi[:], scalar1=1.0, scalar2=0.0,
                        op0=OP.mult, op1=OP.add,
                        accum_out=S_acc[:, 8 + cc * 4 + b:
                                        8 + cc * 4 + b + 1])
                px = sc2.tile([128, BBS], F16, tag="prod_x")
                nc.gpsimd.tensor_tensor(out=px[:], in0=xr, in1=xi,
                                        op=OP.mult)
                nc.vector.tensor_scalar(
                    out=px[:], in0=px[:], scalar1=1.0, scalar2=0.0,
                    op0=OP.mult, op1=OP.add,
                    accum_out=S_acc[:, 16 + cc * 4 + b:
                                    16 + cc * 4 + b + 1])

            def emit_means(b, cc):
                lo = BBS * b
                for comp in range(2):
                    if cc == 0 and comp == 0:
                        prod_i[("pm", b)] = [
                            ps_mean.tile([128, 1024], FP, tag=f"psm{c2}",
                                         name=f"psm{c2}_{b}")
                            for c2 in range(2)]
                    pm = prod_i[("pm", b)]
                    for m in range(4):
                        nc.tensor.matmul(
                            pm[comp][64 * (m % 2):64 * (m % 2) + 1,
                                     512 * (m // 2):512 * (m // 2) + 512],
                            onesF_t,
                            xsl(comp, cc, lo + 512 * m, 512),
                            start=(cc == 0), stop=(cc == NCH - 1))

            def emit_tree(b, cc, comp, deep_pool):
                lo = BBS * b
                t1 = trp.tile([128, BBS // 2], F16, tag="t1")
                nc.gpsimd.tensor_tensor(
                    out=t1[:], in0=xsl(comp, cc, lo, BBS // 2),
                    in1=xsl(comp, cc, lo + BBS // 2, BBS // 2), op=OP.add)
                eng = nc.gpsimd if deep_pool else nc.vector
                t2 = trp.tile([128, BBS // 4], F16, tag="t2")
                eng.tensor_tensor(out=t2[:], in0=t1[:, 0:BBS // 4],
                                  in1=t1[:, BBS // 4:BBS // 2], op=OP.add)
                t3 = trp.tile([128, BBS // 8], F16, tag="t3")
                eng.tensor_tensor(out=t3[:], in0=t2[:, 0:BBS // 8],
                                  in1=t2[:, BBS // 8:BBS // 4], op=OP.add)
                td = T_sb[:, C * (2 * cc + comp):C * (2 * cc + comp) + C]
                if b == 0:
                    eng.tensor_tensor(out=td, in0=t3[:, 0:C],
                                      in1=t3[:, C:2 * C], op=OP.add)
                else:
                    t4 = trp.tile([128, C], F16, tag="t4")
                    eng.tensor_tensor(out=t4[:], in0=t3[:, 0:C],
                                      in1=t3[:, C:2 * C], op=OP.add)
                    eng.tensor_tensor(out=td, in0=td, in1=t4[:], op=OP.add)

            def emit_stage_drain(b):
                pm = prod_i[("pm", b)]
                mt = msp.tile([128, 2048], F8, tag="ms", name=f"ms_{b}")
                nc.scalar.copy(mt[:, 0:1024], pm[0][:])
                nc.scalar.copy(mt[:, 1024:2048], pm[1][:])
                for comp in range(2):
                    srcd = mt[:, 1024 * comp:1024 * (comp + 1)].rearrange(
                        "(m o) (h j) -> m o h j", o=64, h=2)[:, 0:1]
                    dstd = ar_in[comp:comp + 1,
                                 BBS * b:BBS * (b + 1)].rearrange(
                        "o (h m j) -> m o h j", h=2, m=2)
                    nc.sync.dma_start(dstd, srcd)

            def emit_collective():
                nc.gpsimd.collective_compute(
                    "AllGather", OP.bypass,
                    replica_groups=[list(range(NCORES))],
                    ins=[ar_in.opt()],
                    outs=[ar_out.opt()],
                )
                # pass-2 constants ride the collective window
                nc.sync.dma_start(mg[16:17, :], onesrow[:])
                nc.sync.dma_start(Arow[16:17, :], beta_row[:])
                for hf in range(2):
                    nc.sync.dma_start(
                        mg[0:16, 4096 * hf:4096 * (hf + 1)],
                        ar_out[:, 4096 * hf:4096 * (hf + 1)])

            for b in range(NBB):
                last = b == NBB - 1
                for cc in range(NCH):
                    emit_dmas(b, cc)
                    if b == 0 and cc == 0:
                        emit_const_dmas()
                    emit_means(b, cc)
                    if not (last and cc == 1):
                        emit_sq_acc(b, cc, act_only=False)
                if last:
                    emit_sq_acc(b, 1, act_only=True)
                for cc in range(NCH):
                    for comp in range(2):
                        emit_tree(b, cc, comp,
                                  deep_pool=(last and cc == 1 and comp == 1))
                emit_stage_drain(b)
                if last:
                    emit_collective()

            # ---- stats finalize: S6 (cols m*2+cc), corr6, cov6
            S6 = small.tile([128, 6], FP, tag="S6")
            nc.vector.tensor_reduce(
                S6[:, 0:2], S_acc[:, 0:8].rearrange("p (g b) -> p g b", b=4),
                AX.X, OP.add)
            nc.vector.tensor_reduce(
                S6[:, 2:6], S_acc[:, 8:24].rearrange("p (g b) -> p g b", b=4),
                AX.X, OP.add)
            corr6 = small.tile([128, 6], FP, tag="corr6")
            for m, (ca, cb) in enumerate(((0, 0), (1, 1), (0, 1))):
                for cc in range(NCH):
                    pr = sc2.tile([128, C], F16, tag="prod_x",
                                  name=f"tt_{m}_{cc}")
                    nc.vector.scalar_tensor_tensor(
                        out=pr[:],
                        in0=T_sb[:, C * (2 * cc + ca):C * (2 * cc + ca) + C],
                        scalar=1.0,
                        in1=T_sb[:, C * (2 * cc + cb):C * (2 * cc + cb) + C],
                        op0=OP.mult, op1=OP.mult,
                        accum_out=corr6[:, m * 2 + cc:m * 2 + cc + 1])
            cov6 = small.tile([128, 6], FP, tag="cov6")
            nc.vector.scalar_tensor_tensor(
                out=cov6[:], in0=corr6[:], scalar=-1.0 / B, in1=S6[:],
                op0=OP.mult, op1=OP.add)
            nc.vector.tensor_scalar(
                out=cov6[:], in0=cov6[:], scalar1=1.0 / NM1, scalar2=None,
                op0=OP.mult)
            _stk.close()  # release scratch + mean PSUM

            # ---- Phase C: closed-form 2x2 inverse sqrt, fold gamma
            def stile(tag):
                return small.tile([128, NCH], FP, tag=tag, name=tag)

            arr, cii = stile("arr"), stile("cii")
            bri = cov6[:, 4:6]
            nc.vector.tensor_scalar(out=arr[:], in0=cov6[:, 0:2],
                                    scalar1=EPS, scalar2=None, op0=OP.add)
            nc.vector.tensor_scalar(out=cii[:], in0=cov6[:, 2:4],
                                    scalar1=EPS, scalar2=None, op0=OP.add)

            det, tmp = stile("det"), stile("tmp")
            tsum0 = stile("tsum0")
            nc.vector.tensor_tensor(out=tmp[:], in0=bri, in1=bri,
                                    op=OP.mult)
            nc.vector.tensor_tensor(out=det[:], in0=arr[:], in1=cii[:],
                                    op=OP.mult)
            nc.vector.tensor_tensor(out=tsum0[:], in0=arr[:], in1=cii[:],
                                    op=OP.add)
            nc.vector.tensor_tensor(out=det[:], in0=det[:], in1=tmp[:],
                                    op=OP.subtract)
            s_t = stile("s_t")
            nc.scalar.activation(s_t[:], det[:], AF.Sqrt)
            tsum = stile("tsum")
            nc.vector.scalar_tensor_tensor(out=tsum[:], in0=s_t[:], scalar=2.0,
                                           in1=tsum0[:], op0=OP.mult,
                                           op1=OP.add)
            tval = stile("tval")
            nc.scalar.activation(tval[:], tsum[:], AF.Sqrt)
            den, rden = stile("den"), stile("rden")
            nc.vector.tensor_tensor(out=den[:], in0=s_t[:], in1=tval[:],
                                    op=OP.mult)
            nc.vector.reciprocal(rden[:], den[:])

            w_rr, w_ii, wri = stile("w_rr"), stile("w_ii"), stile("wri")
            nc.vector.tensor_tensor(out=w_rr[:], in0=cii[:], in1=s_t[:],
                                    op=OP.add)  # runs parallel to tval chain
            nc.vector.tensor_tensor(out=w_rr[:], in0=w_rr[:], in1=rden[:],
                                    op=OP.mult)
            nc.vector.tensor_tensor(out=w_ii[:], in0=arr[:], in1=s_t[:],
                                    op=OP.add)
            nc.vector.tensor_tensor(out=w_ii[:], in0=w_ii[:], in1=rden[:],
                                    op=OP.mult)
            nc.vector.scalar_tensor_tensor(out=wri[:], in0=bri,
                                           scalar=-1.0, in1=rden[:],
                                           op0=OP.mult, op1=OP.mult)

            # A = G @ W
            a_rr, a_ri = stile("a_rr"), stile("a_ri")
            a_ir, a_ii = stile("a_ir"), stile("a_ii")
            u, v = stile("u"), stile("v")
            for dst, (wa, wb) in ((a_rr, (w_rr, wri)), (a_ri, (wri, w_ii))):
                nc.vector.tensor_tensor(out=u[:], in0=g_r_t, in1=wa[:],
                                        op=OP.mult)
                nc.vector.tensor_tensor(out=v[:], in0=g_i_t, in1=wb[:],
                                        op=OP.mult)
                nc.vector.tensor_tensor(out=dst[:], in0=u[:], in1=v[:],
                                        op=OP.subtract)
            for dst, (wa, wb) in ((a_ir, (w_rr, wri)), (a_ii, (wri, w_ii))):
                nc.vector.tensor_tensor(out=u[:], in0=g_i_t, in1=wa[:],
                                        op=OP.mult)
                nc.vector.tensor_tensor(out=v[:], in0=g_r_t, in1=wb[:],
                                        op=OP.mult)
                nc.vector.tensor_tensor(out=dst[:], in0=u[:], in1=v[:],
                                        op=OP.add)

            # ---- W tiles (fp16), diagonal per chunk: W[p, 2g+c]
            Ws = []
            for cc in range(NCH):
                W_r = small.tile([128, 256], F16, tag=f"W_r{cc}",
                                 name=f"W_r{cc}")
                W_i = small.tile([128, 256], F16, tag=f"W_i{cc}",
                                 name=f"W_i{cc}")
                for W, (ev, od) in ((W_r, (a_rr, a_ir)), (W_i, (a_ri, a_ii))):
                    Wv = W[:].rearrange("p (g c) -> p g c", c=2)
                    nc.vector.tensor_scalar(
                        out=Wv[:, :, 0], in0=ident_t,
                        scalar1=ev[:, cc:cc + 1], scalar2=None, op0=OP.mult)
                    nc.vector.tensor_scalar(
                        out=Wv[:, :, 1], in0=ident_t,
                        scalar1=od[:, cc:cc + 1], scalar2=None, op0=OP.mult)
                Ws.append((W_r, W_i))

            # ---- Arow rows 0..15: -A/MSCALE coefs in apply-column order,
            # via matmuls against the W tiles with even/odd selectors.
            from contextlib import ExitStack as _ES2
            _stk2 = _ES2()
            ps_t = _stk2.enter_context(
                tc.tile_pool(name="ps_t", bufs=1, space="PSUM"))
            psA = ps_t.tile([16, 2 * FSH], FP, tag="psA")
            for cc in range(NCH):
                W_r, W_i = Ws[cc]
                nc.tensor.matmul(psA[:, 256 * cc:256 * (cc + 1)],
                                 onesEv_t, W_r[:], start=True, stop=False)
                nc.tensor.matmul(psA[:, 256 * cc:256 * (cc + 1)],
                                 onesOd_t, W_i[:], start=False, stop=True)
            nc.vector.tensor_scalar(out=Arow[0:16, :], in0=psA[:],
                                    scalar1=-1.0 / MSCALE, scalar2=None,
                                    op0=OP.mult)
            _stk2.close()

            # ---- Phase D: two-pass apply with interleaved emission
            stg = stgp.tile([128, NG * 512], F16, tag="stg")
            _stk3 = _ES2()
            ps_o = _stk3.enter_context(
                tc.tile_pool(name="ps_o", bufs=7, space="PSUM"))
            corrp = _stk3.enter_context(tc.tile_pool(name="corrp", bufs=2))

            def pass1(g):
                po = ps_o.tile([128, 512], FP, tag="po", name=f"po1_{g}")
                for cc in range(NCH):
                    W_r, W_i = Ws[cc]
                    nc.tensor.matmul(
                        po[:, 256 * cc:256 * (cc + 1)],
                        xsl(0, cc, 128 * g, 128), W_r[:],
                        start=True, stop=False)
                    nc.tensor.matmul(
                        po[:, 256 * cc:256 * (cc + 1)],
                        xsl(1, cc, 128 * g, 128), W_i[:],
                        start=False, stop=True)
                dst = stg[:, 512 * g:512 * (g + 1)]
                if g % 2 == 0:
                    nc.vector.tensor_copy(dst, po[:])
                else:
                    nc.scalar.copy(dst, po[:])

            def store4(g):
                g0 = g - 3
                dstd = out.rearrange("(a p) f -> p a f", p=128)[
                    :, g0:g0 + 4, :]
                srcd = stg[:, 512 * g0:512 * (g + 1)].rearrange(
                    "p (a q) -> p a q", q=512)
                nc.sync.dma_start(dstd, srcd)

            def single(g):
                # one-pass group: K17 correction first (full region, start),
                # then the x-matmul sub-region accumulations (baseline's
                # beta-first PSUM pattern).
                po = ps_o.tile([128, 512], FP, tag="po", name=f"po1_{g}")
                nc.tensor.matmul(
                    po[:], mg[:, 128 * g:128 * (g + 1)], Arow[:],
                    start=True, stop=False)
                for cc in range(NCH):
                    W_r, W_i = Ws[cc]
                    nc.tensor.matmul(
                        po[:, 256 * cc:256 * (cc + 1)],
                        xsl(0, cc, 128 * g, 128), W_r[:],
                        start=False, stop=False)
                    nc.tensor.matmul(
                        po[:, 256 * cc:256 * (cc + 1)],
                        xsl(1, cc, 128 * g, 128), W_i[:],
                        start=False, stop=(cc == NCH - 1))
                dst = stg[:, 512 * g:512 * (g + 1)]
                if g % 2 == 0:
                    nc.vector.tensor_copy(dst, po[:])
                else:
                    nc.scalar.copy(dst, po[:])
                if g % 4 == 3:
                    store4(g)

            def pass2(g):
                po = ps_o.tile([128, 512], FP, tag="po", name=f"po2_{g}")
                nc.tensor.matmul(
                    po[:], mg[:, 128 * g:128 * (g + 1)], Arow[:],
                    start=True, stop=True)
                dst = stg[:, 512 * g:512 * (g + 1)]
                if g % 2 == 0 or g >= 28:
                    nc.vector.scalar_tensor_tensor(
                        out=dst, in0=dst, scalar=1.0, in1=po[:],
                        op0=OP.mult, op1=OP.add)
                else:
                    ct = corrp.tile([128, 512], F16, tag="ct")
                    nc.scalar.copy(ct[:], po[:])
                    nc.gpsimd.tensor_tensor(
                        out=dst, in0=dst, in1=ct[:], op=OP.add)
                if g % 4 == 3:
                    store4(g)

            SPLIT = 36
            for g in range(SPLIT):
                pass1(g)
            p2 = 0
            for i, g in enumerate(range(SPLIT, NG)):
                single(g)
                want = (i + 1) * SPLIT // (NG - SPLIT)
                while p2 < min(want, SPLIT):
                    pass2(p2)
                    p2 += 1
            while p2 < SPLIT:
                pass2(p2)
                p2 += 1
            _stk3.close()

    split_multi_waits(nc)
    return nc


_CACHE = {}


def _get_nc():
    if "nc" not in _CACHE:
        _CACHE["nc"] = build_bass()
    return _CACHE["nc"]


def _constants():
    if "consts" not in _CACHE:
        cf16 = np.zeros((128, 40), dtype=np.float16)
        cf16[:, 0] = MSCALE / F
        cf16[:, 8:24] = (np.arange(16) % 2 == 0).astype(np.float16)
        cf16[:, 24:40] = (np.arange(16) % 2 == 1).astype(np.float16)
        _CACHE["consts"] = {
            "cf16": cf16,
            "onesrow": np.ones((1, BC), dtype=ml_dtypes.float8_e4m3),
        }
    return _CACHE["consts"]


def _host_xt(xr, xi, fsl):
    """Build xt[p, 16384*comp + 8192*cc + j] = x_comp[j, 128*cc + p]."""
    halves = []
    for x in (xr, xi):
        xs = x[:, fsl].reshape(BC, NCH, 128)        # (j, cc, p)
        halves.append(np.transpose(xs, (2, 1, 0)).reshape(128, NCH * BC))
    return np.ascontiguousarray(
        np.concatenate(halves, axis=1)).astype(np.float16)


def kernel(x_real, x_imag, gamma_r, gamma_i, beta_r, beta_i):
    x_real = np.asarray(x_real, dtype=np.float32).reshape(BC, F)
    x_imag = np.asarray(x_imag, dtype=np.float32).reshape(BC, F)
    gamma_r = np.asarray(gamma_r, dtype=np.float32)
    gamma_i = np.asarray(gamma_i, dtype=np.float32)
    beta_r = np.asarray(beta_r, dtype=np.float32)
    beta_i = np.asarray(beta_i, dtype=np.float32)

    nc = _get_nc()
    consts = _constants()

    in_maps = []
    for k in range(NCORES):
        fsl = slice(FSH * k, FSH * (k + 1))
        cf32 = np.empty((128, 128 + 2 * NCH), dtype=np.float32)
        cf32[:, 0:128] = np.eye(128, dtype=np.float32)
        cf32[:, 128:128 + NCH] = gamma_r[fsl].reshape(NCH, 128).T
        cf32[:, 128 + NCH:] = gamma_i[fsl].reshape(NCH, 128).T
        beta_row = np.ascontiguousarray(
            np.stack([beta_r[fsl], beta_i[fsl]], axis=-1).reshape(1, 2 * FSH)
        ).astype(ml_dtypes.float8_e4m3)
        in_maps.append({
            "xt": _host_xt(x_real, x_imag, fsl),
            "cf32": cf32, "beta_row": beta_row,
            **consts,
        })

    res = run_bass_kernel_spmd(nc, in_maps, list(range(NCORES)))

    full = np.empty((B, C, F, 2), dtype=np.float32)
    for k in range(NCORES):
        full[:, :, FSH * k:FSH * (k + 1)] = (
            np.asarray(res.results[k]["out"]).astype(np.float32)
            .reshape(B, C, FSH, 2)
        )
    return full
